# revision 8
# baseline (speedup 1.0000x reference)
"""L21 norm kernel for Trainium2 (Bass/Tile), 8-core SPMD.

Computes sum_j sqrt(sum_i S[i,j]^2) for S of shape [8192, 16384] fp32.

Sharding: S is split along columns into 8 shards of [8192, 2048] (one per
NeuronCore). Each core computes the per-column L2 norms of its 2048
columns and DMAs the [1, 2048] norm vector out; the host sums all norms
in float64.

Per-core dataflow (memory-bound; 64 MiB HBM read per core; mid-stream
DMA measured at ~431 GB/s = the SBUF-AXI fabric ceiling, so all wins are
at the stream edges):
  - Bulk: 14 tiles of [128 partitions, 4 rows, 2048 cols] fp32 (4 MiB
    HWDGE DMAs; each partition's slice is 32 KiB contiguous in DRAM).
    ACT squares each tile (bf16 out, also the PE dtype cast); per tile,
    row-slices q=0,1 reduce on PE (ones[128,1]^T @ sq into a [1,2048]
    fp32 PSUM row) and q=2,3 accumulate on DVE into a bf16 [128,2048]
    accumulator (folded into PSUM every 5 tiles to keep bf16 chains
    short).
  - Tail (rows 7168..8191): 8 slices of [128, 1, 2048] so the ACT
    backlog after the last byte is bounded by one 2 us slice square
    instead of a 7 us tile square. Even slices reduce on PE, odd slices
    feed the DVE accumulator, which is pre-folded (no stop) before the
    last slice arrives.
  - Last slice: four [128, 512] column-quarter DMAs; each quarter is
    squared on DVE (tensor_mul, ACT stays free for sqrts) and consumed
    directly by that block's stop-matmul, then its [1, 512] sqrt chunk
    fires. Post-last-byte chain: one DVE mul -> one matmul -> one sqrt.
  - Epilogue: one 8 KiB DMA of the [1, 2048] norms; host sums in f64.
"""

import numpy as np

# Full problem shape (hardcoded per the harness contract).
R = 8192          # rows
C_FULL = 16384    # columns
N_CORES = 8
C = C_FULL // N_CORES  # 2048 columns per core
P = 128           # SBUF partitions
NBLK = 512        # matmul moving free dim (one PSUM bank of fp32)

T4 = 14           # bulk tiles: [P, 4, C], rows 0..7168
ROWS4 = T4 * P * 4
NT = 8            # tail slices: [P, 1, C], rows 7168..8191
# DVE-accumulator fold points (after the adds at tile t) and the tiles
# that restart the accumulator with a copy. The last stretch (tiles
# 10-13 plus the odd tail slices) folds once in the tail, pre-quarters.
FOLD_TILES = (4, 9)
RESET_TILES = (5, 10)
NQ = 4            # column quarters of the final tail slice
QW = C // NQ      # 512 columns per quarter

_cached = None


def _build():
    """Build + schedule the per-core Bass program. Returns the Bacc object."""
    import concourse.bacc as bacc
    import concourse.tile as tile
    from concourse import mybir

    nc = bacc.Bacc(
        "TRN2",
        target_bir_lowering=False,
        debug=False,
        enable_asserts=False,
        num_devices=N_CORES,
    )

    s_dram = nc.dram_tensor("S", [R, C], mybir.dt.float32, kind="ExternalInput")
    out_dram = nc.dram_tensor("out", [1, C], mybir.dt.float32, kind="ExternalOutput")

    s_ap = s_dram.ap()
    out_ap = out_dram.ap()

    # Bulk view [T4, P, 4, C]: partition p holds 4 consecutive rows ->
    # 32 KiB contiguous DRAM per (t, p) descriptor.
    v4 = s_ap[:ROWS4, :].rearrange("(t p q) c -> t p q c", p=P, q=4)
    # Tail: eight [P, C] row-slices (1 MiB each; 8 KiB per partition).
    v1 = s_ap[ROWS4:, :].rearrange("(s p) c -> s p c", p=P)

    with tile.TileContext(nc) as tc:
        with (
            tc.tile_pool(name="io", bufs=3) as io_pool,
            tc.tile_pool(name="sqp", bufs=3) as sq_pool,
            tc.tile_pool(name="tio", bufs=4) as tio_pool,
            tc.tile_pool(name="tsq", bufs=3) as tsq_pool,
            tc.tile_pool(name="qsq", bufs=4) as qsq_pool,
            tc.tile_pool(name="const", bufs=1) as const_pool,
            tc.tile_pool(name="ps", bufs=1, space="PSUM") as ps_pool,
            tc.tile_pool(name="fin", bufs=1) as fin_pool,
        ):
            # First input DMA before any const setup so streaming starts as
            # early as possible.
            x0 = io_pool.tile([P, 4, C], mybir.dt.float32, tag="x")
            # Issued from the ACT engine's HWDGE ring: its preamble clears
            # earlier than Sync's, so streaming starts sooner.
            nc.scalar.dma_start(out=x0, in_=v4[0])

            ones = const_pool.tile([P, 1], mybir.dt.bfloat16)
            nc.vector.memset(ones, 1.0)

            # DVE-side accumulator for q=2,3 row-slices / odd tail slices.
            acc = const_pool.tile([P, C], mybir.dt.bfloat16)

            # Per-column sum of squares (4 PSUM banks).
            colsq = ps_pool.tile([1, C], mybir.dt.float32)

            # Dummy sqrt: pulls the sqrt ACT-table load out of the tail.
            warm = const_pool.tile([1, 1], mybir.dt.float32)
            nc.scalar.sqrt(out=warm, in_=ones[0:1, :])

            def pe_reduce(src, first=False, blocks=range(C // NBLK), stop_blocks=()):
                for b in blocks:
                    nc.tensor.matmul(
                        colsq[:, b * NBLK : (b + 1) * NBLK],
                        ones,
                        src[:, b * NBLK : (b + 1) * NBLK],
                        start=first,
                        stop=(b in stop_blocks),
                    )

            for t in range(T4):
                if t == 0:
                    x_tile = x0
                else:
                    x_tile = io_pool.tile([P, 4, C], mybir.dt.float32, tag="x")
                    nc.sync.dma_start(out=x_tile, in_=v4[t])

                sq = sq_pool.tile([P, 4, C], mybir.dt.bfloat16, tag="sq")
                nc.scalar.square(out=sq, in_=x_tile)

                pe_reduce(sq[:, 0, :], first=(t == 0))
                pe_reduce(sq[:, 1, :])

                if t == 0 or t in RESET_TILES:
                    nc.vector.tensor_copy(acc, sq[:, 2, :])
                else:
                    nc.vector.tensor_add(acc, acc, sq[:, 2, :])
                nc.vector.tensor_add(acc, acc, sq[:, 3, :])

                if t in FOLD_TILES:
                    pe_reduce(acc)

            # Tail slices 0..6: slice-granular DMAs through a 4-deep pool
            # (DMA s+4 gates on square s, which ACT finishes well before
            # the buffer is needed again). Even slices go to PE, odd
            # slices to the DVE accumulator.
            for s in range(NT - 1):
                xs = tio_pool.tile([P, 1, C], mybir.dt.float32, tag="xt")
                nc.sync.dma_start(out=xs[:, 0, :], in_=v1[s])
                sqs = tsq_pool.tile([P, 1, C], mybir.dt.bfloat16, tag="sqt")
                nc.scalar.square(out=sqs, in_=xs)
                if s % 2 == 0:
                    pe_reduce(sqs[:, 0, :])
                else:
                    nc.vector.tensor_add(acc, acc, sqs[:, 0, :])

            # Pre-fold the accumulator (bulk tiles 10-13 + odd tail
            # slices) before the last slice arrives; stops come from the
            # per-quarter matmuls below.
            pe_reduce(acc)

            # Last slice as four column quarters: DVE squares them (ACT
            # stays free for the sqrt chunks) and each feeds its block's
            # stop-matmul directly.
            xq = tio_pool.tile([P, 1, C], mybir.dt.float32, tag="xt")
            norms = fin_pool.tile([1, C], mybir.dt.float32)
            sq_q = [
                qsq_pool.tile([P, QW], mybir.dt.bfloat16, tag="qsq", name=f"sq_q{q}")
                for q in range(NQ)
            ]
            for q in range(NQ):
                cols = slice(q * QW, (q + 1) * QW)
                nc.sync.dma_start(out=xq[:, 0, cols], in_=v1[NT - 1][:, cols])
                nc.vector.tensor_mul(sq_q[q], xq[:, 0, cols], xq[:, 0, cols])
                nc.tensor.matmul(
                    colsq[:, cols], ones, sq_q[q], start=False, stop=True
                )
                nc.scalar.activation(
                    norms[:, cols], colsq[:, cols],
                    mybir.ActivationFunctionType.Sqrt,
                )

            nc.sync.dma_start(out=out_ap, in_=norms)

    nc.compile()
    return nc


def _get_nc():
    global _cached
    if _cached is None:
        _cached = _build()
    return _cached


def _run(S: np.ndarray, trace: bool = False):
    from concourse import bass_utils

    assert S.shape == (R, C_FULL), S.shape
    S = np.ascontiguousarray(np.asarray(S, dtype=np.float32))

    nc = _get_nc()
    in_maps = [
        {"S": np.ascontiguousarray(S[:, i * C : (i + 1) * C])} for i in range(N_CORES)
    ]
    try:
        res = bass_utils.run_bass_kernel_spmd(
            nc, in_maps, core_ids=list(range(N_CORES)), trace=trace
        )
    except Exception:
        # One retry: transient NRT/device hiccups (e.g. a wedged core from a
        # previous process) are recoverable on re-execution.
        res = bass_utils.run_bass_kernel_spmd(
            nc, in_maps, core_ids=list(range(N_CORES)), trace=trace
        )
    partials = np.array(
        [np.asarray(res.results[i]["out"], dtype=np.float64).sum() for i in range(N_CORES)],
        dtype=np.float64,
    )
    out = np.float32(partials.sum())
    return out, res


def kernel(S: np.ndarray) -> np.ndarray:
    out, _ = _run(S, trace=False)
    return np.asarray(out, dtype=np.float32)


def run_traced(S: np.ndarray):
    """For test.py: returns (output, BassKernelResults) with NTFF trace."""
    return _run(S, trace=True)
